# revision 9
# baseline (speedup 1.0000x reference)
"""CRF loss (forward-algorithm logsumexp recurrence) on 8 NeuronCores.

Strategy: data-parallel over batch (B=128 -> 16 per core). The forward
recurrence runs in the exp domain: with P_t = exp(state_t - offs_t),
    P_t = (P_{t-1} @ E) * F_t,   E = exp(transitions), F_t = exp(f_t - c_t)
so each step is two 128-contraction matmuls (state kept transposed as
[k, b] so the constant E tiles are the stationary operand) plus one
elementwise multiply. Per-step normalizers c_t (host-computed from the
feature frames) keep P in a tight dynamic range, so fp16 operands are
safe. A ones-vector matmul accumulates s_t[b] = sum_k P_t[k, b] every
step into an on-chip log; the host picks s at t = len[b]-1, takes the
log, re-adds the accumulated normalizers, and subtracts the gold-path
score (a cheap O(B*T) gather done on host).
"""

import numpy as np

B, T, K = 128, 256, 256
N_CORES = 8
BL = B // N_CORES  # batch per core
KT = K // 128      # k tiles (contraction/output splits)
CHUNK = 32         # timesteps of F per DMA chunk

_cache = {}


def _build_nc():
    from contextlib import ExitStack

    import concourse.bacc as bacc
    import concourse.tile as tile
    from concourse import mybir

    nc = bacc.Bacc("TRN2", target_bir_lowering=False, debug=False,
                   num_devices=N_CORES)
    f16 = mybir.dt.float16
    bf16 = mybir.dt.bfloat16
    f32 = mybir.dt.float32

    e_in = nc.dram_tensor("e_in", [KT, KT, 128, 128], bf16,
                          kind="ExternalInput").ap()
    # F[k, t*2*BL + j*BL + b] = exp(f[t, b, 128j + k] - c_t)
    f_in = nc.dram_tensor("f_in", [128, T * 2 * BL], f32,
                          kind="ExternalInput").ap()
    s_out = nc.dram_tensor("s_out", [1, T * 2 * BL], f32,
                           kind="ExternalOutput").ap()

    WIN = 4          # P' slots per state buffer; s-sum matmul per window
    with tile.TileContext(nc) as tc, ExitStack() as ctx:
        consts = ctx.enter_context(tc.tile_pool(name="consts", bufs=1))
        fpool = ctx.enter_context(tc.tile_pool(name="fpool", bufs=2))
        state = ctx.enter_context(tc.tile_pool(name="state", bufs=2))
        psum = ctx.enter_context(tc.tile_pool(name="psum", bufs=2,
                                              space="PSUM"))
        psum_s = ctx.enter_context(tc.tile_pool(name="psum_s", bufs=2,
                                                space="PSUM"))

        # constants: E tiles + ones column
        e_t = [[consts.tile([128, 128], bf16, tag=f"e{i}{j}", name=f"e{i}{j}")
                for j in range(KT)] for i in range(KT)]
        for i in range(KT):
            for j in range(KT):
                nc.sync.dma_start(e_t[i][j][:], e_in[i, j, :, :])
        ones = consts.tile([128, 1], bf16, tag="ones", name="ones")
        nc.vector.memset(ones[:], 1.0)

        # s log: one row, all timesteps x (j, b) partials
        s_buf = consts.tile([1, T * 2 * BL], f32, tag="sbuf", name="sbuf")

        n_chunks = T // CHUNK
        fch = [None] * n_chunks

        def load_chunk(c):
            ft = fpool.tile([128, CHUNK * 2 * BL], f32, tag="f", name="fch")
            nc.sync.dma_start(
                ft[:], f_in[:, c * CHUNK * 2 * BL:(c + 1) * CHUNK * 2 * BL])
            fch[c] = ft

        load_chunk(0)
        load_chunk(1)

        # P' tiles: WIN step-slots of 32 cols in one [128, WIN*32] buffer,
        # double-buffered by window. Column layout per slot: j*BL + b.
        pbuf_prev = None
        pbuf = state.tile([128, WIN * 2 * BL], f16, tag="pb", name="pb")
        for t in range(T):
            c, r = divmod(t, CHUNK)
            if r == 0 and c + 2 < n_chunks:
                load_chunk(c + 2)
            w, slot = divmod(t, WIN)
            fcol = fch[c][:, r * 2 * BL:(r + 1) * 2 * BL]   # [128, 32]
            p_new = pbuf[:, slot * 2 * BL:(slot + 1) * 2 * BL]
            if t == 0:
                nc.vector.tensor_copy(p_new, fcol)
            else:
                pt = t - 1
                pslot = pt % WIN
                src = pbuf_prev if pslot == WIN - 1 else pbuf
                p_prev = src[:, pslot * 2 * BL:(pslot + 1) * 2 * BL]
                ps = psum.tile([128, 2 * BL], f32, tag="ps", name="ps")
                nc.tensor.matmul(ps[:, 0:BL], e_t[0][0][:],
                                 p_prev[:, 0:BL], start=True, stop=False)
                nc.tensor.matmul(ps[:, 0:BL], e_t[1][0][:],
                                 p_prev[:, BL:2 * BL], start=False, stop=True)
                nc.tensor.matmul(ps[:, BL:2 * BL], e_t[0][1][:],
                                 p_prev[:, 0:BL], start=True, stop=False)
                nc.tensor.matmul(ps[:, BL:2 * BL], e_t[1][1][:],
                                 p_prev[:, BL:2 * BL], start=False, stop=True)
                nc.vector.tensor_mul(p_new, ps[:], fcol)
            if slot == WIN - 1 or t == T - 1:
                # s partials for the whole window in one matmul
                ncols = (slot + 1) * 2 * BL
                pss = psum_s.tile([1, WIN * 2 * BL], f32, tag="pss",
                                  name="pss")
                nc.tensor.matmul(pss[:, 0:ncols], ones[:], pbuf[:, 0:ncols],
                                 start=True, stop=True)
                nc.scalar.copy(
                    s_buf[:, w * WIN * 2 * BL:w * WIN * 2 * BL + ncols],
                    pss[:, 0:ncols])
                pbuf_prev = pbuf
                pbuf = state.tile([128, WIN * 2 * BL], f16, tag="pb",
                                  name="pb")

        nc.sync.dma_start(s_out[:], s_buf[:])

    nc.compile()
    return nc


def _prepare(feats, transitions, feats_len):
    f = np.ascontiguousarray(feats.transpose(1, 0, 2)).astype(np.float32)
    # per-step normalizer: mean over batch of logsumexp_k of the frame
    m = f.max(axis=2)
    lse = np.log(np.exp(f - m[:, :, None]).sum(axis=2,
                                               dtype=np.float32)) + m
    c = lse.mean(axis=1).astype(np.float32)             # [T]
    offs = np.cumsum(c.astype(np.float64))              # [T]

    import ml_dtypes
    E = np.exp(transitions.astype(np.float32))
    e_packed = np.empty((KT, KT, 128, 128), ml_dtypes.bfloat16)
    for i in range(KT):
        for j in range(KT):
            e_packed[i, j] = E[128 * i:128 * (i + 1),
                               128 * j:128 * (j + 1)].astype(ml_dtypes.bfloat16)

    # F[t, b, k] = exp(f[t, b, k] - c_t) -> [128, T*2*BL] per core
    Fx = np.exp(f - c[:, None, None]).astype(np.float32)  # [T, B, K]
    f_maps = []
    for core in range(N_CORES):
        sl = Fx[:, core * BL:(core + 1) * BL, :]          # [T, BL, K]
        blk = sl.reshape(T, BL, KT, 128).transpose(3, 0, 2, 1)
        f_maps.append({"f_in": np.ascontiguousarray(
            blk.reshape(128, T * KT * BL))})
    return e_packed, f_maps, offs, c


def _gold_score(feats, transitions, tags, feats_len):
    f = feats.transpose(1, 0, 2).astype(np.float32)       # [T, B, K]
    tg = tags.T.astype(np.int64)                          # [T, B]
    mask = (np.arange(T)[:, None] < feats_len[None, :])
    maskf = mask.astype(np.float32)
    emit = np.take_along_axis(f, tg[:, :, None], axis=2)[:, :, 0] * maskf
    u = emit.sum(axis=0, dtype=np.float32)
    t_mask = maskf[:-1] * maskf[1:]
    t_score = transitions.astype(np.float32)[tg[:-1], tg[1:]] * t_mask
    return (u + t_score.sum(axis=0, dtype=np.float32)).astype(np.float32)


def kernel(feats, transitions, tags, feats_len, _results_hook=None,
           _trace=False):
    from concourse.bass_utils import run_bass_kernel_spmd

    feats = np.asarray(feats, dtype=np.float32)
    transitions = np.asarray(transitions, dtype=np.float32)
    tags_np = np.asarray(tags)
    feats_len_np = np.asarray(feats_len).astype(np.int64)

    if "nc" not in _cache:
        _cache["nc"] = _build_nc()
    nc = _cache["nc"]

    e_packed, f_maps, offs, _c = _prepare(feats, transitions, feats_len_np)
    in_maps = [{"e_in": e_packed, **f_maps[core]} for core in range(N_CORES)]

    res = run_bass_kernel_spmd(nc, in_maps, core_ids=list(range(N_CORES)),
                               trace=_trace)
    if _results_hook is not None:
        _results_hook(res)

    u = _gold_score(feats, transitions, tags_np, feats_len_np)
    loss = np.empty(B, np.float32)
    idx = feats_len_np - 1                                 # [B] capture step
    for core in range(N_CORES):
        s = res.results[core]["s_out"].reshape(T, KT, BL).astype(
            np.float64).sum(axis=1)                        # [T, BL]
        bl = np.arange(BL)
        bg = core * BL + bl
        sv = s[idx[bg], bl]
        loss[bg] = (np.log(sv) + offs[idx[bg]]).astype(np.float32) - u[bg]
    return loss


# revision 11
# speedup vs baseline: 1.1054x; 1.1054x over previous
"""CRF loss (forward-algorithm logsumexp recurrence) on 8 NeuronCores.

Strategy: data-parallel over batch (B=128 -> 16 per core). The forward
recurrence runs in the exp domain: with P_t = exp(state_t - offs_t),
    P_t = (P_{t-1} @ E) * F_t,   E = exp(transitions), F_t = exp(f_t - c_t)
so each step is two 128-contraction matmuls (state kept transposed as
[k, b] so the constant E tiles are the stationary operand) plus one
elementwise multiply. Per-step normalizers c_t (host-computed from the
feature frames) keep P in a tight dynamic range, so fp16 operands are
safe. A ones-vector matmul accumulates s_t[b] = sum_k P_t[k, b] every
step into an on-chip log; the host picks s at t = len[b]-1, takes the
log, re-adds the accumulated normalizers, and subtracts the gold-path
score (a cheap O(B*T) gather done on host).
"""

import numpy as np

B, T, K = 128, 256, 256
N_CORES = 8
BL = B // N_CORES  # batch per core
KT = K // 128      # k tiles (contraction/output splits)
CHUNK = 32         # timesteps of F per DMA chunk

_cache = {}


def _build_nc():
    from contextlib import ExitStack

    import concourse.bacc as bacc
    import concourse.tile as tile
    from concourse import mybir

    nc = bacc.Bacc("TRN2", target_bir_lowering=False, debug=False,
                   num_devices=N_CORES)
    f16 = mybir.dt.float16
    bf16 = mybir.dt.bfloat16
    f32 = mybir.dt.float32

    e_in = nc.dram_tensor("e_in", [KT, KT, 128, 128], bf16,
                          kind="ExternalInput").ap()
    # F[k, t*2*BL + j*BL + b] = exp(f[t, b, 128j + k] - c_t)
    f_in = nc.dram_tensor("f_in", [128, T * 2 * BL], f32,
                          kind="ExternalInput").ap()
    s_out = nc.dram_tensor("s_out", [1, T * 2 * BL], f32,
                           kind="ExternalOutput").ap()

    WIN = 8          # P' slots per state buffer; s-sum matmul per window
    with tile.TileContext(nc) as tc, ExitStack() as ctx:
        consts = ctx.enter_context(tc.tile_pool(name="consts", bufs=1))
        fpool = ctx.enter_context(tc.tile_pool(name="fpool", bufs=2))
        state = ctx.enter_context(tc.tile_pool(name="state", bufs=2))
        psum = ctx.enter_context(tc.tile_pool(name="psum", bufs=2,
                                              space="PSUM"))
        psum_s = ctx.enter_context(tc.tile_pool(name="psum_s", bufs=2,
                                                space="PSUM"))

        # constants: E tiles + ones column
        e_t = [[consts.tile([128, 128], bf16, tag=f"e{i}{j}", name=f"e{i}{j}")
                for j in range(KT)] for i in range(KT)]
        for i in range(KT):
            for j in range(KT):
                nc.sync.dma_start(e_t[i][j][:], e_in[i, j, :, :])
        ones = consts.tile([128, 1], bf16, tag="ones", name="ones")
        nc.vector.memset(ones[:], 1.0)

        # s log: one row, all timesteps x (j, b) partials
        s_buf = consts.tile([1, T * 2 * BL], f32, tag="sbuf", name="sbuf")

        n_chunks = T // CHUNK
        fch = [None] * n_chunks

        def load_chunk(c):
            ft = fpool.tile([128, CHUNK * 2 * BL], f32, tag="f", name="fch")
            nc.sync.dma_start(
                ft[:], f_in[:, c * CHUNK * 2 * BL:(c + 1) * CHUNK * 2 * BL])
            fch[c] = ft

        load_chunk(0)
        load_chunk(1)

        # P' tiles: WIN step-slots of 32 cols in one [128, WIN*32] buffer,
        # double-buffered by window. Column layout per slot: j*BL + b.
        pbuf_prev = None
        pbuf = state.tile([128, WIN * 2 * BL], f16, tag="pb", name="pb")
        for t in range(T):
            c, r = divmod(t, CHUNK)
            if r == 0 and c + 2 < n_chunks:
                load_chunk(c + 2)
            w, slot = divmod(t, WIN)
            fcol = fch[c][:, r * 2 * BL:(r + 1) * 2 * BL]   # [128, 32]
            p_new = pbuf[:, slot * 2 * BL:(slot + 1) * 2 * BL]
            if t == 0:
                nc.vector.tensor_copy(p_new, fcol)
            else:
                pt = t - 1
                pslot = pt % WIN
                src = pbuf_prev if pslot == WIN - 1 else pbuf
                p_prev = src[:, pslot * 2 * BL:(pslot + 1) * 2 * BL]
                # two PSUM banks so the DVE can multiply half 0 while the
                # PE is still writing half 1 (same-bank PE-W/DVE-R would
                # serialize)
                ps0 = psum.tile([128, BL], f32, tag="ps0", name="ps0")
                ps1 = psum.tile([128, BL], f32, tag="ps1", name="ps1")
                nc.tensor.matmul(ps0[:], e_t[0][0][:],
                                 p_prev[:, 0:BL], start=True, stop=False)
                nc.tensor.matmul(ps0[:], e_t[1][0][:],
                                 p_prev[:, BL:2 * BL], start=False, stop=True)
                nc.tensor.matmul(ps1[:], e_t[0][1][:],
                                 p_prev[:, 0:BL], start=True, stop=False)
                nc.tensor.matmul(ps1[:], e_t[1][1][:],
                                 p_prev[:, BL:2 * BL], start=False, stop=True)
                nc.vector.tensor_mul(p_new[:, 0:BL], ps0[:], fcol[:, 0:BL])
                nc.vector.tensor_mul(p_new[:, BL:2 * BL], ps1[:],
                                     fcol[:, BL:2 * BL])
            if slot == WIN - 1 or t == T - 1:
                # s partials for the whole window in one matmul
                ncols = (slot + 1) * 2 * BL
                pss = psum_s.tile([1, WIN * 2 * BL], f32, tag="pss",
                                  name="pss")
                nc.tensor.matmul(pss[:, 0:ncols], ones[:], pbuf[:, 0:ncols],
                                 start=True, stop=True)
                nc.scalar.copy(
                    s_buf[:, w * WIN * 2 * BL:w * WIN * 2 * BL + ncols],
                    pss[:, 0:ncols])
                pbuf_prev = pbuf
                pbuf = state.tile([128, WIN * 2 * BL], f16, tag="pb",
                                  name="pb")

        nc.sync.dma_start(s_out[:], s_buf[:])

    nc.compile()
    return nc


def _prepare(feats, transitions, feats_len):
    f = np.ascontiguousarray(feats.transpose(1, 0, 2)).astype(np.float32)
    # per-step normalizer: mean over batch of logsumexp_k of the frame
    m = f.max(axis=2)
    lse = np.log(np.exp(f - m[:, :, None]).sum(axis=2,
                                               dtype=np.float32)) + m
    c = lse.mean(axis=1).astype(np.float32)             # [T]
    offs = np.cumsum(c.astype(np.float64))              # [T]

    import ml_dtypes
    E = np.exp(transitions.astype(np.float32))
    e_packed = np.empty((KT, KT, 128, 128), ml_dtypes.bfloat16)
    for i in range(KT):
        for j in range(KT):
            e_packed[i, j] = E[128 * i:128 * (i + 1),
                               128 * j:128 * (j + 1)].astype(ml_dtypes.bfloat16)

    # F[t, b, k] = exp(f[t, b, k] - c_t) -> [128, T*2*BL] per core
    Fx = np.exp(f - c[:, None, None]).astype(np.float32)  # [T, B, K]
    f_maps = []
    for core in range(N_CORES):
        sl = Fx[:, core * BL:(core + 1) * BL, :]          # [T, BL, K]
        blk = sl.reshape(T, BL, KT, 128).transpose(3, 0, 2, 1)
        f_maps.append({"f_in": np.ascontiguousarray(
            blk.reshape(128, T * KT * BL))})
    return e_packed, f_maps, offs, c


def _gold_score(feats, transitions, tags, feats_len):
    f = feats.transpose(1, 0, 2).astype(np.float32)       # [T, B, K]
    tg = tags.T.astype(np.int64)                          # [T, B]
    mask = (np.arange(T)[:, None] < feats_len[None, :])
    maskf = mask.astype(np.float32)
    emit = np.take_along_axis(f, tg[:, :, None], axis=2)[:, :, 0] * maskf
    u = emit.sum(axis=0, dtype=np.float32)
    t_mask = maskf[:-1] * maskf[1:]
    t_score = transitions.astype(np.float32)[tg[:-1], tg[1:]] * t_mask
    return (u + t_score.sum(axis=0, dtype=np.float32)).astype(np.float32)


def kernel(feats, transitions, tags, feats_len, _results_hook=None,
           _trace=False):
    from concourse.bass_utils import run_bass_kernel_spmd

    feats = np.asarray(feats, dtype=np.float32)
    transitions = np.asarray(transitions, dtype=np.float32)
    tags_np = np.asarray(tags)
    feats_len_np = np.asarray(feats_len).astype(np.int64)

    if "nc" not in _cache:
        _cache["nc"] = _build_nc()
    nc = _cache["nc"]

    e_packed, f_maps, offs, _c = _prepare(feats, transitions, feats_len_np)
    in_maps = [{"e_in": e_packed, **f_maps[core]} for core in range(N_CORES)]

    res = run_bass_kernel_spmd(nc, in_maps, core_ids=list(range(N_CORES)),
                               trace=_trace)
    if _results_hook is not None:
        _results_hook(res)

    u = _gold_score(feats, transitions, tags_np, feats_len_np)
    loss = np.empty(B, np.float32)
    idx = feats_len_np - 1                                 # [B] capture step
    for core in range(N_CORES):
        s = res.results[core]["s_out"].reshape(T, KT, BL).astype(
            np.float64).sum(axis=1)                        # [T, BL]
        bl = np.arange(BL)
        bg = core * BL + bl
        sv = s[idx[bg], bl]
        loss[bg] = (np.log(sv) + offs[idx[bg]]).astype(np.float32) - u[bg]
    return loss


# revision 14
# speedup vs baseline: 1.2075x; 1.0924x over previous
"""CRF loss (forward-algorithm logsumexp recurrence) on 8 NeuronCores.

Strategy: data-parallel over batch (B=128 -> 16 per core). The forward
recurrence runs in the exp domain: with P_t = exp(state_t - offs_t),
    P_t = (P_{t-1} @ E) * F_t,   E = exp(transitions), F_t = exp(f_t - c_t)
so each step is two 128-contraction matmuls (state kept transposed as
[k, b] so the constant E tiles are the stationary operand) plus one
elementwise multiply. Per-step normalizers c_t (host-computed from the
feature frames) keep P in a tight dynamic range, so fp16 operands are
safe. A ones-vector matmul accumulates s_t[b] = sum_k P_t[k, b] every
step into an on-chip log; the host picks s at t = len[b]-1, takes the
log, re-adds the accumulated normalizers, and subtracts the gold-path
score (a cheap O(B*T) gather done on host).
"""

import numpy as np

B, T, K = 128, 256, 256
N_CORES = 8
BL = B // N_CORES  # batch per core
KT = K // 128      # k tiles (contraction/output splits)
CHUNK = 32         # timesteps of F per DMA chunk

_cache = {}


def _build_nc():
    from contextlib import ExitStack

    import concourse.bacc as bacc
    import concourse.tile as tile
    from concourse import mybir

    nc = bacc.Bacc("TRN2", target_bir_lowering=False, debug=False,
                   num_devices=N_CORES)
    f16 = mybir.dt.float16
    bf16 = mybir.dt.bfloat16
    f32 = mybir.dt.float32

    e_in = nc.dram_tensor("e_in", [KT, KT, 128, 128], bf16,
                          kind="ExternalInput").ap()
    # F[k, t*2*BL + j*BL + b] = exp(f[t, b, 128j + k] - c_t)
    f_in = nc.dram_tensor("f_in", [128, T * 2 * BL], f32,
                          kind="ExternalInput").ap()
    s_out = nc.dram_tensor("s_out", [1, T * 2 * BL], f32,
                           kind="ExternalOutput").ap()

    WIN = 8          # P' slots per state buffer; s-sum matmul per window
    with tile.TileContext(nc) as tc, ExitStack() as ctx:
        consts = ctx.enter_context(tc.tile_pool(name="consts", bufs=1))
        fpool = ctx.enter_context(tc.tile_pool(name="fpool", bufs=2))
        state = ctx.enter_context(tc.tile_pool(name="state", bufs=2))
        psum = ctx.enter_context(tc.tile_pool(name="psum", bufs=2,
                                              space="PSUM"))
        psum_s = ctx.enter_context(tc.tile_pool(name="psum_s", bufs=2,
                                                space="PSUM"))

        n_chunks = T // CHUNK
        fch = [None] * n_chunks

        def load_chunk(c):
            ft = fpool.tile([128, CHUNK * 2 * BL], f32, tag="f", name="fch")
            nc.sync.dma_start(
                ft[:], f_in[:, c * CHUNK * 2 * BL:(c + 1) * CHUNK * 2 * BL])
            fch[c] = ft

        # chunk 0 first: step 0 only needs it (E tiles wait until step 1)
        load_chunk(0)

        # constants: E tiles + ones column
        e_t = [[consts.tile([128, 128], bf16, tag=f"e{i}{j}", name=f"e{i}{j}")
                for j in range(KT)] for i in range(KT)]
        for i in range(KT):
            for j in range(KT):
                nc.sync.dma_start(e_t[i][j][:], e_in[i, j, :, :])
        ones = consts.tile([128, 1], bf16, tag="ones", name="ones")
        nc.vector.memset(ones[:], 1.0)

        # s log: one row, all timesteps x (j, b) partials
        s_buf = consts.tile([1, T * 2 * BL], f32, tag="sbuf", name="sbuf")

        load_chunk(1)

        def emit_s(w, pb, ncols):
            pss = psum_s.tile([1, WIN * 2 * BL], f32, tag="pss", name="pss")
            nc.tensor.matmul(pss[:, 0:ncols], ones[:], pb[:, 0:ncols],
                             start=True, stop=True)
            nc.scalar.copy(
                s_buf[:, w * WIN * 2 * BL:w * WIN * 2 * BL + ncols],
                pss[:, 0:ncols])

        # P' tiles: WIN step-slots of 32 cols in one [128, WIN*32] buffer,
        # double-buffered by window. Column layout per slot: j*BL + b.
        pbuf_prev = None
        pending_s = None
        pbuf = state.tile([128, WIN * 2 * BL], f16, tag="pb", name="pb")
        for t in range(T):
            c, r = divmod(t, CHUNK)
            if r == 0 and c + 2 < n_chunks:
                load_chunk(c + 2)
            w, slot = divmod(t, WIN)
            fcol = fch[c][:, r * 2 * BL:(r + 1) * 2 * BL]   # [128, 32]
            p_new = pbuf[:, slot * 2 * BL:(slot + 1) * 2 * BL]
            if t == 0:
                nc.vector.tensor_copy(p_new, fcol)
            else:
                pt = t - 1
                pslot = pt % WIN
                src = pbuf_prev if pslot == WIN - 1 else pbuf
                p_prev = src[:, pslot * 2 * BL:(pslot + 1) * 2 * BL]
                # two PSUM banks so the DVE can multiply half 0 while the
                # PE is still writing half 1 (same-bank PE-W/DVE-R would
                # serialize)
                ps0 = psum.tile([128, BL], f32, tag="ps0", name="ps0")
                ps1 = psum.tile([128, BL], f32, tag="ps1", name="ps1")
                nc.tensor.matmul(ps0[:], e_t[0][0][:],
                                 p_prev[:, 0:BL], start=True, stop=False)
                nc.tensor.matmul(ps0[:], e_t[1][0][:],
                                 p_prev[:, BL:2 * BL], start=False, stop=True)
                nc.tensor.matmul(ps1[:], e_t[0][1][:],
                                 p_prev[:, 0:BL], start=True, stop=False)
                nc.tensor.matmul(ps1[:], e_t[1][1][:],
                                 p_prev[:, BL:2 * BL], start=False, stop=True)
                nc.vector.tensor_mul(p_new[:, 0:BL], ps0[:], fcol[:, 0:BL])
                nc.vector.tensor_mul(p_new[:, BL:2 * BL], ps1[:],
                                     fcol[:, BL:2 * BL])
            if pending_s is not None and slot == 1:
                emit_s(*pending_s)
                pending_s = None
            if slot == WIN - 1 or t == T - 1:
                # s partials for the whole window in one matmul; deferred
                # to early next window so it fills a PE idle gap instead
                # of blocking the next step's matmuls in the PE FIFO
                pending_s = (w, pbuf, (slot + 1) * 2 * BL)
                pbuf_prev = pbuf
                pbuf = state.tile([128, WIN * 2 * BL], f16, tag="pb",
                                  name="pb")

        if pending_s is not None:
            emit_s(*pending_s)

        nc.sync.dma_start(s_out[:], s_buf[:])

    nc.compile()
    return nc


def _prepare(feats, transitions, feats_len):
    f = np.ascontiguousarray(feats.transpose(1, 0, 2)).astype(np.float32)
    # per-step normalizer: mean over batch of logsumexp_k of the frame
    m = f.max(axis=2)
    lse = np.log(np.exp(f - m[:, :, None]).sum(axis=2,
                                               dtype=np.float32)) + m
    c = lse.mean(axis=1).astype(np.float32)             # [T]
    offs = np.cumsum(c.astype(np.float64))              # [T]

    import ml_dtypes
    E = np.exp(transitions.astype(np.float32))
    e_packed = np.empty((KT, KT, 128, 128), ml_dtypes.bfloat16)
    for i in range(KT):
        for j in range(KT):
            e_packed[i, j] = E[128 * i:128 * (i + 1),
                               128 * j:128 * (j + 1)].astype(ml_dtypes.bfloat16)

    # F[t, b, k] = exp(f[t, b, k] - c_t) -> [128, T*2*BL] per core
    Fx = np.exp(f - c[:, None, None]).astype(np.float32)  # [T, B, K]
    f_maps = []
    for core in range(N_CORES):
        sl = Fx[:, core * BL:(core + 1) * BL, :]          # [T, BL, K]
        blk = sl.reshape(T, BL, KT, 128).transpose(3, 0, 2, 1)
        f_maps.append({"f_in": np.ascontiguousarray(
            blk.reshape(128, T * KT * BL))})
    return e_packed, f_maps, offs, c


def _gold_score(feats, transitions, tags, feats_len):
    f = feats.transpose(1, 0, 2).astype(np.float32)       # [T, B, K]
    tg = tags.T.astype(np.int64)                          # [T, B]
    mask = (np.arange(T)[:, None] < feats_len[None, :])
    maskf = mask.astype(np.float32)
    emit = np.take_along_axis(f, tg[:, :, None], axis=2)[:, :, 0] * maskf
    u = emit.sum(axis=0, dtype=np.float32)
    t_mask = maskf[:-1] * maskf[1:]
    t_score = transitions.astype(np.float32)[tg[:-1], tg[1:]] * t_mask
    return (u + t_score.sum(axis=0, dtype=np.float32)).astype(np.float32)


def kernel(feats, transitions, tags, feats_len, _results_hook=None,
           _trace=False):
    from concourse.bass_utils import run_bass_kernel_spmd

    feats = np.asarray(feats, dtype=np.float32)
    transitions = np.asarray(transitions, dtype=np.float32)
    tags_np = np.asarray(tags)
    feats_len_np = np.asarray(feats_len).astype(np.int64)

    if "nc" not in _cache:
        _cache["nc"] = _build_nc()
    nc = _cache["nc"]

    e_packed, f_maps, offs, _c = _prepare(feats, transitions, feats_len_np)
    in_maps = [{"e_in": e_packed, **f_maps[core]} for core in range(N_CORES)]

    res = run_bass_kernel_spmd(nc, in_maps, core_ids=list(range(N_CORES)),
                               trace=_trace)
    if _results_hook is not None:
        _results_hook(res)

    u = _gold_score(feats, transitions, tags_np, feats_len_np)
    loss = np.empty(B, np.float32)
    idx = feats_len_np - 1                                 # [B] capture step
    for core in range(N_CORES):
        s = res.results[core]["s_out"].reshape(T, KT, BL).astype(
            np.float64).sum(axis=1)                        # [T, BL]
        bl = np.arange(BL)
        bg = core * BL + bl
        sv = s[idx[bg], bl]
        loss[bg] = (np.log(sv) + offs[idx[bg]]).astype(np.float32) - u[bg]
    return loss
